# revision 1
# baseline (speedup 1.0000x reference)
"""Trainium2 Bass kernel for windowed multi-head attention with conv QKV.

Shapes (hardcoded): x (2,64,32,192), D_MODEL=32, N_HEADS=8, c=4, QS=24,
FLANGE=8, F=40, T=192, M=8 blocks. 8 NeuronCores.

Sharding: core k owns batch b=k//4 and block pair m0=2*(k%4); it computes
all 8 heads for its two blocks (16 attention groups) plus its slice of the
final conv (second launch).
"""

import numpy as np
import ml_dtypes
import concourse.bass as bass
import concourse.bacc as bacc
import concourse.mybir as mybir
from concourse.tile import TileContext
from concourse.bass_utils import run_bass_kernel_spmd
from concourse.tile_rust import add_dep_helper

F32 = mybir.dt.float32
F32R = mybir.dt.float32r
U32 = mybir.dt.uint32
AF = mybir.ActivationFunctionType

NCORES = 8
B, CIN, H, W = 2, 64, 32, 192
DM, NH, CH = 32, 8, 4          # d_model, heads, depth/head
QS, FL, F = 24, 8, 40          # query block, flange, window
M = W // QS                    # 8 blocks
POS = B * H * W                # 12288
PB = H * W                     # 6144 positions per batch
NPAD = B * (H + 2) * W         # 13056 (h-padded raster)
ROWP = (H + 2) * W             # 6528
KFSZ = B * NH * CH * H * (W + 2 * FL)   # 425984 padded k storage
# buggy as_strided strides (elements) over the padded storage
S_B, S_N, S_C, S_H = NH * CH * H * W, CH * H * W, H * W, W  # 196608,24576,6144,192
HF = H * F                     # 1280 keys per group
HQ = H * QS                    # 768 queries per group

_CACHE = {}


def _sap(tile, p0, npart, off, dims):
    """Custom strided view of an SBUF/PSUM pool tile.

    Partition step comes from the tile's own AP (accounts for allocation
    padding); `off` is a free-dim element offset, `dims` the free dims.
    """
    b0 = tile[:]
    ps = int(b0.ap[0][0])
    return bass.AP(b0.tensor, b0.offset + p0 * ps + off, [[ps, npart]] + dims)



def _build_launch1():
    nc = bacc.Bacc(None, target_bir_lowering=False, debug=False,
                   num_devices=NCORES)
    x = nc.dram_tensor("x", [B, CIN, H, W], F32R, kind="ExternalInput").ap()
    # blob cols: wA0..2 @ dy*96, wB0..2 @ 288+dy*96, bias @576, id4 @577-581,
    # zf zeros @ 608..992
    wblob = nc.dram_tensor("wblob", [128, 992], F32R,
                           kind="ExternalInput").ap()
    zpad = nc.dram_tensor("zpad", [B, 2, 32, 32, 8], F32R,
                          kind="ExternalInput").ap()
    v2init = nc.dram_tensor("v2init", [128, 330], mybir.dt.bfloat16,
                            kind="ExternalInput").ap()
    par = nc.dram_tensor("par", [1, 4], U32, kind="ExternalInput").ap()
    o_out = nc.dram_tensor("o_out", [4, NH * 2 * H * QS], F32,
                           kind="ExternalOutput").ap()

    kf = nc.dram_tensor("kf", [KFSZ], F32R).ap()
    vf = nc.dram_tensor("vf", [KFSZ], F32R).ap()
    qfull = nc.dram_tensor("qfull", [DM, POS], F32R).ap()

    with TileContext(nc) as tc:
      with tc.tile_pool(name="persist", bufs=1) as persist:
        # ---------------- conv phase ----------------
        with (
            tc.tile_pool(name="xw", bufs=1) as xw,
            tc.tile_pool(name="ystage", bufs=1) as ystage,
            tc.tile_pool(name="cps", bufs=2, space="PSUM") as cps,
        ):
            par_sb = persist.tile([1, 4], U32, tag="par", name="par")
            nc.sync.dma_start(out=par_sb[:], in_=par[:])
            blob = persist.tile([128, 992], F32R, tag="blob", name="blob")
            nc.sync.dma_start(out=blob[:], in_=wblob[:])
            wA_sb = [blob[:, dy * 96:(dy + 1) * 96] for dy in range(3)]
            wB_sb = [blob[0:64, 288 + dy * 96:288 + (dy + 1) * 96]
                     for dy in range(3)]
            bias_sb = blob[0:96, 576:577].bitcast(F32)
            id_sb = blob[0:4, 577:581]

            # X2: rows 0-63 dx=-1, rows 64-127 dx=0 ; X1: rows 0-63 dx=+1,
            # row 64 ones (bias). free = (b, hh in [0,34), w)
            X2 = xw.tile([128, NPAD], F32R, tag="X2", name="X2")
            X1 = xw.tile([64, NPAD], F32R, tag="X1", name="X1")
            # load x once, then build shifted copies on Pool/DVE
            x_sb = xw.tile([64, POS], F32R, tag="x_sb", name="x_sb")
            CH2 = PB // 2
            for b in range(B):
                xb = x[b]
                for c in range(2):
                    nc.sync.dma_start(
                        out=x_sb[:, b * PB + c * CH2: b * PB + (c + 1) * CH2],
                        in_=bass.AP(xb.tensor, xb.offset + c * CH2,
                                    [[PB, 64], [1, CH2]]))
            def _xcopy(eng, dst_t, p0, doff, soff, n1):
                if eng is nc.scalar:
                    nc.scalar.activation(
                        _sap(dst_t, p0, 64, doff, [[1, n1]]),
                        _sap(x_sb, 0, 64, soff, [[1, n1]]), AF.Identity)
                else:
                    eng.tensor_copy(
                        _sap(dst_t, p0, 64, doff, [[1, n1]]),
                        _sap(x_sb, 0, 64, soff, [[1, n1]]))

            for b in range(B):
                eng = nc.vector if b == 0 else nc.scalar
                for c in range(2):
                    s0 = c * CH2
                    n1 = CH2 if c == 0 else CH2 - 1
                    _xcopy(eng, X2, 0, b * ROWP + W + 1 + s0, b * PB + s0, n1)
                    _xcopy(eng, X2, 64, b * ROWP + W + s0, b * PB + s0, CH2)
                    _xcopy(eng, X1, 0, b * ROWP + W + s0, b * PB + s0 + 1, n1)
            # then zero pad rows (hh=0,33) and shifted edge cols (overwrite)
            for hh_off in (0, (H + 1) * W):
                nc.sync.dma_start(
                    out=_sap(X2, 0, 128, hh_off, [[ROWP, 2], [1, W]]),
                    in_=_sap(blob, 0, 128, 608, [[W, 2], [1, W]]))
                nc.sync.dma_start(
                    out=_sap(X1, 0, 64, hh_off, [[ROWP, 2], [1, W]]),
                    in_=_sap(blob, 0, 64, 608, [[W, 2], [1, W]]))
            nc.sync.dma_start(
                out=_sap(X2, 0, 64, 0, [[W, 2 * (H + 2)], [1, 1]]),
                in_=_sap(blob, 0, 64, 608, [[1, 2 * (H + 2)], [1, 1]]))
            nc.sync.dma_start(
                out=_sap(X1, 0, 64, W - 1, [[W, 2 * (H + 2)], [1, 1]]),
                in_=_sap(blob, 0, 64, 608, [[1, 2 * (H + 2)], [1, 1]]))

            Y = ystage.tile([96, POS], F32R, tag="Y", name="Y")
            kf_writes, vf_writes, qf_writes = [], [], []
            NT = 512
            W2 = W + 2 * FL   # 208 storage row
            for b in range(B):
                for ct in range(PB // NT):
                    yp = cps.tile([96, NT], F32, tag="yp", name="yp")
                    p0 = ct * NT
                    for dy in range(3):
                        off = b * ROWP + dy * W + p0
                        nc.tensor.matmul(
                            yp[:], wA_sb[dy],
                            _sap(X2, 0, 128, off, [[1, NT]]),
                            start=(dy == 0), stop=False)
                    for dy in range(3):
                        off = b * ROWP + dy * W + p0
                        nc.tensor.matmul(
                            yp[:], wB_sb[dy],
                            _sap(X1, 0, 64, off, [[1, NT]]),
                            start=False, stop=(dy == 2))
                    dst = b * PB + p0
                    if b == 0:
                        nc.scalar.activation(Y[:, dst:dst + NT], yp[:],
                                             AF.Identity, bias=bias_sb)
                    else:
                        nc.vector.tensor_scalar_add(Y[:, dst:dst + NT], yp[:],
                                                    bias_sb)
                # flush this batch's conv outputs to DRAM immediately
                qf_writes.append(nc.sync.dma_start(
                    out=bass.AP(qfull.tensor, b * PB, [[POS, 32], [1, PB]]),
                    in_=Y[0:32, b * PB:(b + 1) * PB]).ins)
                for (prow, dst, wl) in ((32, kf, kf_writes), (64, vf, vf_writes)):
                    for hh0 in (0, 16):
                        wl.append(nc.sync.dma_start(
                            out=bass.AP(dst.tensor,
                                        b * 32 * H * W2 + hh0 * W2 + FL,
                                        [[H * W2, 32], [W2, 16], [1, W]]),
                            in_=Y[prow:prow + 32,
                                  b * PB + hh0 * W:b * PB + (hh0 + 16) * W]
                            .rearrange("p (h w) -> p h w", h=16)).ins)
                    for fi, fo in ((0, 0), (1, W2 - FL)):
                        zp = zpad[b, fi]  # (32, 32, 8)
                        wl.append(nc.sync.dma_start(
                            out=bass.AP(dst.tensor, b * 32 * H * W2 + fo,
                                        [[H * W2, 32], [W2, H], [1, FL]]),
                            in_=bass.AP(zp.tensor, zp.offset,
                                        [[H * FL, 32], [FL, H], [1, FL]])).ins)

            # base registers for dynamic gathers (SP engine)
            r0 = nc.gpsimd.alloc_register("cb")
            nc.gpsimd.reg_load(r0, par_sb[0:1, 0:1])
            cbase = nc.snap(r0, min_val=0, max_val=300000)
            r1 = nc.gpsimd.alloc_register("qb")
            nc.gpsimd.reg_load(r1, par_sb[0:1, 1:2])
            qbase = nc.snap(r1, min_val=0, max_val=300000)


        # ---------------- attention phase ----------------
        with (
            tc.tile_pool(name="ga", bufs=2) as ga,
            tc.tile_pool(name="ste", bufs=3) as stp,
            tc.tile_pool(name="ops", bufs=1, space="PSUM") as ops,
            tc.tile_pool(name="pst", bufs=2, space="PSUM") as pst,
            tc.tile_pool(name="on", bufs=2) as on,
            tc.tile_pool(name="dsc", bufs=24, space="DRAM") as dsc,
        ):
            o_all = on.tile([4, NH * 2 * HQ], F32, tag="o_all", name="o_all",
                            bufs=1)
            # persistent v2e staging (ones at col t*33+32, zeros elsewhere)
            v2e = [ga.tile([128, 330], mybir.dt.bfloat16, tag=f"v2e{i}", name=f"v2e{i}") for i in range(2)]
            for t in v2e:
                nc.sync.dma_start(out=t[:], in_=v2init[:])

            KVLEN = 3 * S_C + 31 * S_H + QS + F   # 24640 strip (covers mm=0,1)
            QLEN = 3 * POS + 31 * W + QS + QS      # q strip (covers mm=0,1)
            kscrs, vscrs, qscrs = [], [], []
            for n in range(NH):
                gb = cbase + n * S_N
                qb = qbase + n * CH * POS
                kscr = dsc.tile([KVLEN], F32R, tag=f"kscr{n}", name=f"kscr{n}")
                vscr = dsc.tile([KVLEN], F32R, tag=f"vscr{n}", name=f"vscr{n}")
                qscr = dsc.tile([QLEN], F32R, tag=f"qscr{n}", name=f"qscr{n}")
                kscrs.append(kscr); vscrs.append(vscr); qscrs.append(qscr)
                i_k = nc.gpsimd.dma_start(
                    out=kscr[:], in_=bass.AP(kf.tensor, gb, [[1, KVLEN]]))
                i_v = nc.gpsimd.dma_start(
                    out=vscr[:], in_=bass.AP(vf.tensor, gb, [[1, KVLEN]]))
                i_q = nc.gpsimd.dma_start(
                    out=qscr[:], in_=bass.AP(qfull.tensor, qb, [[1, QLEN]]))
                for inst, wl in ((i_k, kf_writes), (i_v, vf_writes),
                                 (i_q, qf_writes)):
                    for wi in wl:
                        add_dep_helper(inst.ins, wi, sync=True,
                                       reason="gather strip after conv flush")
            for n in range(NH):
                kscr, vscr, qscr = kscrs[n], vscrs[n], qscrs[n]
                for mm in range(2):
                    g = n * 2 + mm

                    k2 = ga.tile([4, HF], F32R, tag="k2", name="k2")
                    nc.sync.dma_start(
                        out=k2[:],
                        in_=bass.AP(kscr.tensor, kscr.offset + mm * QS,
                                    [[S_C, 4], [S_H, H], [1, F]]))
                    v2k = ga.tile([4, HF], F32R, tag="v2k", name="v2k")
                    nc.gpsimd.dma_start(
                        out=v2k[:],
                        in_=bass.AP(vscr.tensor, vscr.offset + mm * QS,
                                    [[S_C, 4], [S_H, H], [1, F]]))
                    qg = ga.tile([4, HQ], F32R, tag="qg", name="qg")
                    nc.sync.dma_start(
                        out=qg[:],
                        in_=bass.AP(qscr.tensor, qscr.offset + mm * QS,
                                    [[POS, 4], [W, H], [1, QS]]))

                    # transpose v2k -> (128,40) per 128-key tile, pack v2e
                    ve = v2e[g % 2]
                    vt = pst.tile([128, 40], F32R, tag="stq", name="stq")
                    for kt in range(10):
                        nc.tensor.transpose(
                            vt[:, kt * 4:(kt + 1) * 4],
                            v2k[:, kt * 128:(kt + 1) * 128], id_sb)
                    nc.vector.tensor_copy(
                        _sap(ve, 0, 128, 0, [[33, 10], [1, 4]]),
                        _sap(vt, 0, 128, 0, [[4, 10], [1, 4]]))

                    oA = ops.tile([33, 512], F32, tag="oA", name="oA")
                    oB = ops.tile([33, 256], F32, tag="oB", name="oB")
                    for st_i in range(5):
                        st = pst.tile([128, 1536], F32, tag="stq", name="stq")
                        # bank-aligned writes: [512@0, 256@512, 256@768, 512@1024]
                        kt0 = st_i * 2
                        kt1 = kt0 + 1
                        nc.tensor.matmul(
                            st[:, 0:512], k2[:, kt0 * 128:(kt0 + 1) * 128],
                            qg[:, 0:512],
                            start=True, stop=True, skip_group_check=True)
                        nc.tensor.matmul(
                            st[:, 512:768], k2[:, kt0 * 128:(kt0 + 1) * 128],
                            qg[:, 512:768],
                            start=True, stop=True, skip_group_check=True)
                        nc.tensor.matmul(
                            st[:, 768:1024], k2[:, kt1 * 128:(kt1 + 1) * 128],
                            qg[:, 0:256],
                            start=True, stop=True, skip_group_check=True)
                        nc.tensor.matmul(
                            st[:, 1024:1536], k2[:, kt1 * 128:(kt1 + 1) * 128],
                            qg[:, 256:768],
                            start=True, stop=True, skip_group_check=True)
                        ste = stp.tile([128, 1536], mybir.dt.bfloat16, tag="ste", name="ste")
                        nc.scalar.activation(ste[:], st[:], AF.Exp)
                        for half in range(2):
                            kt = st_i * 2 + half
                            c0 = half * 768
                            lhs = _sap(ve, 0, 128, kt * 33, [[1, 33]])
                            nc.tensor.matmul(
                                oA[:], lhs, ste[:, c0:c0 + 512],
                                start=(kt == 0), stop=(kt == 9),
                                skip_group_check=True)
                            nc.tensor.matmul(
                                oB[:], lhs, ste[:, c0 + 512:c0 + 768],
                                start=(kt == 0), stop=(kt == 9),
                                skip_group_check=True)

                    osb = on.tile([33, HQ], F32, tag="osb", name="osb")
                    nc.vector.tensor_copy(osb[:, 0:512], oA[:])
                    nc.vector.tensor_copy(osb[:, 512:768], oB[:])
                    rec = on.tile([1, HQ], F32, tag="rec", name="rec")
                    nc.vector.reciprocal(rec[:], osb[32:33, :])
                    rec4 = on.tile([4, HQ], F32, tag="rec4", name="rec4")
                    nc.gpsimd.partition_broadcast(rec4[:], rec[:])
                    nc.vector.tensor_mul(o_all[:, g * HQ:(g + 1) * HQ],
                                         osb[0:4, :], rec4[:])

            nc.sync.dma_start(out=o_out[:], in_=o_all[:])
    nc.finalize()
    return nc


def _build_launch2():
    nc = bacc.Bacc(None, target_bir_lowering=False, debug=False,
                   num_devices=NCORES)
    WH = 2 * QS + 2  # 50 cols with halo
    oh = nc.dram_tensor("oh", [DM, H + 2, WH], F32R, kind="ExternalInput").ap()
    w2 = [nc.dram_tensor(f"w2{dy}", [96, 64], F32R, kind="ExternalInput").ap()
          for dy in range(3)]
    z32 = nc.dram_tensor("z32", [32, 1], F32R, kind="ExternalInput").ap()
    out = nc.dram_tensor("out", [64, H * 2 * QS], F32,
                         kind="ExternalOutput").ap()
    NPAD2 = (H + 2) * WH  # 1700

    with TileContext(nc) as tc:
        with (
            tc.tile_pool(name="sb", bufs=1) as sb,
            tc.tile_pool(name="ps", bufs=2, space="PSUM") as ps,
        ):
            w2_sb = [sb.tile([96, 64], F32R, tag=f"w2{dy}", name=f"w2{dy}sb") for dy in range(3)]
            for dy in range(3):
                nc.sync.dma_start(out=w2_sb[dy][:], in_=w2[dy][:])
            osb = sb.tile([32, NPAD2], F32R, tag="osb", name="osb")
            nc.sync.dma_start(out=osb[:], in_=oh[:].rearrange("c h w -> c (h w)"))
            osh = sb.tile([96, NPAD2], F32R, tag="osh", name="osh")
            nc.sync.dma_start(out=osh[0:32, 0:1], in_=z32[:])
            nc.sync.dma_start(out=osh[64:96, NPAD2 - 1:NPAD2], in_=z32[:])
            nc.vector.tensor_copy(osh[0:32, 1:NPAD2], osb[:, 0:NPAD2 - 1])
            nc.vector.tensor_copy(osh[32:64, :], osb[:])
            nc.vector.tensor_copy(osh[64:96, 0:NPAD2 - 1], osb[:, 1:NPAD2])

            ot = sb.tile([64, H * 2 * QS], F32, tag="ot", name="ot")
            hsz = [10, 10, 10, 2]
            h0 = 0
            for hi, hn in enumerate(hsz):
                nt = hn * WH
                yp = ps.tile([64, 500], F32, tag="yp", name="yp")
                for dy in range(3):
                    off = (h0 + dy) * WH
                    nc.tensor.matmul(
                        yp[:, 0:nt], w2_sb[dy][:],
                        _sap(osh, 0, 96, off, [[1, nt]]),
                        start=(dy == 0), stop=(dy == 2))
                nc.vector.tensor_copy(
                    _sap(ot, 0, 64, h0 * 2 * QS, [[2 * QS, hn], [1, 2 * QS]]),
                    _sap(yp, 0, 64, 1, [[WH, hn], [1, 2 * QS]]))
                h0 += hn
            nc.sync.dma_start(out=out[:], in_=ot[:])
    nc.finalize()
    return nc


def _round_f32r(a):
    return a.astype(np.float32)


def _prep_qkv_weights(q_w, q_b, k_w, k_b, v_w, v_b):
    # fold attention scale into q
    sc = CH ** -0.5
    q_w = q_w * sc
    q_b = q_b * sc
    Wc = np.concatenate([q_w, k_w, v_w], axis=0)   # (96, 64, 3, 3)
    bc = np.concatenate([q_b, k_b, v_b], axis=0)   # (96,)
    wA, wB = [], []
    for dy in range(3):
        a = np.zeros((128, 96), np.float32)
        a[0:64, :] = Wc[:, :, dy, 0].T    # dx=-1
        a[64:128, :] = Wc[:, :, dy, 1].T  # dx=0
        wA.append(a)
        b = Wc[:, :, dy, 2].T.copy()    # dx=+1
        wB.append(b)
    return wA, wB, bc.reshape(96, 1)


def kernel(x, q_w, q_b, k_w, k_b, v_w, v_b, out_w):
    x = np.asarray(x, np.float32)
    if "l1" not in _CACHE:
        _CACHE["l1"] = _build_launch1()
        _CACHE["l2"] = _build_launch2()
    nc1, nc2 = _CACHE["l1"], _CACHE["l2"]

    wA, wB, cbias = _prep_qkv_weights(np.asarray(q_w, np.float32), np.asarray(q_b, np.float32),
                               np.asarray(k_w, np.float32), np.asarray(k_b, np.float32),
                               np.asarray(v_w, np.float32), np.asarray(v_b, np.float32))
    zpad = np.zeros((B, 2, 32, 32, 8), np.float32)
    wblob = np.zeros((128, 992), np.float32)
    for dy in range(3):
        wblob[:, dy * 96:(dy + 1) * 96] = wA[dy]
        wblob[0:64, 288 + dy * 96:288 + (dy + 1) * 96] = wB[dy]
    wblob[0:96, 576] = cbias[:, 0]
    wblob[0:4, 577:581] = np.eye(4, dtype=np.float32)
    v2init = np.zeros((128, 330), ml_dtypes.bfloat16)
    v2init[:, 32::33] = 1.0
    in_maps = []
    for k in range(NCORES):
        b, m0 = k // 4, 2 * (k % 4)
        par = np.array([[b * S_B + m0 * QS, b * PB + m0 * QS, 0, 0]], np.uint32)
        m = {"x": x, "zpad": zpad, "par": par, "wblob": wblob,
             "v2init": v2init}
        in_maps.append(m)
    res1 = run_bass_kernel_spmd(nc1, in_maps, list(range(NCORES)))

    # assemble o (B, 32, H, W)
    o = np.zeros((B, DM, H, W), np.float32)
    for k in range(NCORES):
        b, m0 = k // 4, 2 * (k % 4)
        oo = res1.results[k]["o_out"].reshape(4, NH, 2, H, QS)
        o[b, :, :, m0 * QS:(m0 + 2) * QS] = (
            oo.transpose(1, 0, 3, 2, 4).reshape(DM, H, 2 * QS))

    # launch 2: output conv, sharded by (b, column pair)
    w2 = []
    ow = np.asarray(out_w, np.float32)
    for dy in range(3):
        a = np.zeros((96, 64), np.float32)
        for dx in range(3):
            a[dx * 32:(dx + 1) * 32, :] = ow[:, :, dy, dx].T
        w2.append(a)
    in_maps2 = []
    WH = 2 * QS + 2
    for k in range(NCORES):
        b, m0 = k // 4, 2 * (k % 4)
        ohal = np.zeros((DM, H + 2, WH), np.float32)
        c0 = m0 * QS
        lo, hi = max(0, c0 - 1), min(W, c0 + 2 * QS + 1)
        ohal[:, 1:H + 1, (lo - (c0 - 1)):(hi - (c0 - 1))] = o[b, :, :, lo:hi]
        mm = {"oh": ohal, "z32": np.zeros((32, 1), np.float32)}
        for dy in range(3):
            mm[f"w2{dy}"] = w2[dy]
        in_maps2.append(mm)
    res2 = run_bass_kernel_spmd(nc2, in_maps2, list(range(NCORES)))

    out = np.zeros((B, 64, H, W), np.float32)
    for k in range(NCORES):
        b, m0 = k // 4, 2 * (k % 4)
        out[b, :, :, m0 * QS:(m0 + 2) * QS] = \
            res2.results[k]["out"].reshape(64, H, 2 * QS)
    return out



# revision 21
# speedup vs baseline: 1.2046x; 1.2046x over previous
"""Trainium2 Bass kernel for windowed multi-head attention with conv QKV.

Shapes (hardcoded): x (2,64,32,192), D_MODEL=32, N_HEADS=8, c=4, QS=24,
FLANGE=8, F=40, T=192, M=8 blocks. 8 NeuronCores.

Sharding: core k owns batch b=k//4 and block pair m0=2*(k%4); it computes
all 8 heads for its two blocks (16 attention groups) plus its slice of the
final conv (second launch).

Launch 1 structure:
  conv phase: x landed in a zero-padded 194-wide raster (Xp = X2[0:64]);
    X2[64:128] = Xp shifted by +1 col. 6 matmul passes per chunk
    (3 dy taps x {128-part packed dx pair, 64-part dx}), bias added during
    the PSUM->SBUF flush copy, then static DMAs write kf/vf (padded
    208-wide storage) and qfull to DRAM.
  attention phase: per group (head n, block mm): dynamic gpsimd gathers
    k2/v2k/qg straight from kf/vf/qfull (buggy-stride windows, offset
    register cbase/qbase = per-core (b, m0) base). Scores: 20 matmuls
    into PSUM [128,1536] (f32r, keys x queries). exp split: Act engine
    (exact) for 4/5 tiles, DVE Schraudolph int-trick for 1/5. Weighted
    sum reoriented: out[128 queries, 4v+Z] with exp'd scores as the
    stationary and bf16 (v | ones) as the 5-col moving operand,
    accumulated over the 10 key tiles into a persistent PSUM bank
    (one 30-col region per group). Host does the final divide by Z.
"""

import numpy as np
import ml_dtypes
import concourse.bass as bass
import concourse.bacc as bacc
import concourse.mybir as mybir
from concourse.tile import TileContext
from concourse.bass_utils import run_bass_kernel_spmd
from concourse.tile_rust import add_dep_helper

F32 = mybir.dt.float32
F32R = mybir.dt.float32r
I32 = mybir.dt.int32
U32 = mybir.dt.uint32
BF16 = mybir.dt.bfloat16
AF = mybir.ActivationFunctionType
ALU = mybir.AluOpType

NCORES = 8
B, CIN, H, W = 2, 64, 32, 192
DM, NH, CH = 32, 8, 4          # d_model, heads, depth/head
QS, FL, F = 24, 8, 40          # query block, flange, window
M = W // QS                    # 8 blocks
POS = B * H * W                # 12288
PB = H * W                     # 6144 positions per batch
W2 = W + 2 * FL                # 208 storage row
KFSZ = B * DM * H * W2         # 425984 padded k storage (both batches)
# buggy as_strided strides (elements) over the padded storage
S_B, S_N, S_C, S_H = NH * CH * H * W, CH * H * W, H * W, W
HF = H * F                     # 1280 keys per group
HQ = H * QS                    # 768 queries per group
WP = W + 2                     # 194 padded conv raster row
ROWP = (H + 2) * WP            # 6596 raster per batch
NT = 384                       # conv chunk (2 h rows)
NCT = PB // NT                 # 16 chunks per batch
FLB = 4                        # chunks per flush block (8 h rows)
NG = 2 * NH                    # 16 groups per core

# Schraudolph exp constants (round-to-nearest on the f32->i32 convert)
SCH_A = float(2 ** 23 / np.log(2))
SCH_B = float(127 * 2 ** 23 - 366393.0)

_CACHE = {}
DEBUG_DUMPS = False


def _sap(tile, p0, npart, off, dims):
    """Custom strided view of an SBUF/PSUM pool tile."""
    b0 = tile[:]
    ps = int(b0.ap[0][0])
    return bass.AP(b0.tensor, b0.offset + p0 * ps + off, [[ps, npart]] + dims)


def _build_launch1():
    nc = bacc.Bacc(None, target_bir_lowering=False, debug=False,
                   num_devices=NCORES)
    # full x (conv must cover both batches: groups read across the
    # batch boundary via the buggy strides)
    xb = nc.dram_tensor("xb", [B, CIN, H, W], F32R,
                        kind="ExternalInput").ap()
    # blob cols: wA0..2 @ dy*96, wB0..2 @ 288+dy*96 (rows 0:64), bias @576,
    # id4 @ rows 4:8 cols 577:581, ones-bf16 @ col 584 (128 rows)
    wblob = nc.dram_tensor("wblob", [128, 640], F32R,
                           kind="ExternalInput").ap()
    zpad = nc.dram_tensor("zpad", [2, 32, 32, 16], F32R,
                          kind="ExternalInput").ap()
    par = nc.dram_tensor("par", [1, 4], U32, kind="ExternalInput").ap()
    o_out = nc.dram_tensor("o_out", [128, NG * 30], F32,
                           kind="ExternalOutput").ap()

    kf = nc.dram_tensor("kf", [KFSZ], F32R).ap()
    vf = nc.dram_tensor("vf", [KFSZ], F32R).ap()
    qfull = nc.dram_tensor("qfull", [DM, POS], F32R).ap()

    with TileContext(nc) as tc:
      with tc.tile_pool(name="persist", bufs=1) as persist:
        kf_writes, vf_writes, qf_writes = [], [], []
        # ---------------- conv phase ----------------
        with (
            tc.tile_pool(name="xw", bufs=1) as xw,
            tc.tile_pool(name="yst", bufs=3) as yst,
            tc.tile_pool(name="cps", bufs=2, space="PSUM") as cps,
        ):
            par_sb = persist.tile([1, 4], U32, tag="par", name="par")
            nc.sync.dma_start(out=par_sb[:], in_=par[:])
            blob = persist.tile([128, 640], F32R, tag="blob", name="blob")
            nc.sync.dma_start(out=blob[:], in_=wblob[:])
            wA_sb = [blob[:, dy * 96:(dy + 1) * 96] for dy in range(3)]
            wB_sb = [blob[0:64, 288 + dy * 96:288 + (dy + 1) * 96]
                     for dy in range(3)]
            bias_sb = blob[0:96, 576:577].bitcast(F32)
            id_sb = blob[0:4, 577:581]

            # X2 [128, ROWP]: rows 0:64 = padded raster Xp of own batch
            # (row r=h+1 holds [0, x_h, 0]); rows 64:128 = Xp shifted +1.
            X2 = xw.tile([128, 2 * ROWP], F32R, tag="X2", name="X2")
            # per batch raster at b*ROWP: zero pad rows 0 and 33, pad cols
            for b in range(B):
                r0 = b * ROWP
                nc.sync.dma_start(
                    out=_sap(X2, 0, 128, r0, [[1, WP]]),
                    in_=bass.AP(zpad.tensor, 0, [[255, 128], [1, WP]]))
                nc.sync.dma_start(
                    out=_sap(X2, 0, 128, r0 + (H + 1) * WP, [[1, WP]]),
                    in_=bass.AP(zpad.tensor, 0, [[255, 128], [1, WP]]))
                nc.sync.dma_start(
                    out=_sap(X2, 0, 128, r0 + WP, [[WP, H], [1, 1]]),
                    in_=bass.AP(zpad.tensor, 0,
                                [[255, 128], [1, H], [1, 1]]))
                nc.sync.dma_start(
                    out=_sap(X2, 0, 128, r0 + WP + W + 1, [[WP, H], [1, 1]]),
                    in_=bass.AP(zpad.tensor, 0,
                                [[255, 128], [1, H], [1, 1]]))
                nc.sync.dma_start(
                    out=_sap(X2, 0, 64, r0 + WP + 1, [[WP, H], [1, W]]),
                    in_=xb[b].rearrange("c h w -> c (h w)"))
                # X2[64:128] = raster shifted by one column
                HCH = ROWP // 2
                for c in range(2):
                    s0 = r0 + c * HCH
                    n1 = HCH if (c == 0 or b == 0) else HCH - 1
                    nc.vector.tensor_copy(
                        _sap(X2, 64, 64, s0, [[1, n1]]),
                        _sap(X2, 0, 64, s0 + 1, [[1, n1]]))

            # registers for dynamic gathers (gpsimd)
            r0 = nc.gpsimd.alloc_register("cb")
            nc.gpsimd.reg_load(r0, par_sb[0:1, 0:1])
            cbase = nc.snap(r0, min_val=0, max_val=300000)
            r1 = nc.gpsimd.alloc_register("qb")
            nc.gpsimd.reg_load(r1, par_sb[0:1, 1:2])
            qbase = nc.snap(r1, min_val=0, max_val=300000)

            # flange zeros in kf/vf, per batch & tensor:
            #   lead 8 of row 0 (per ch), 16-wide run at col 200 of rows
            #   0..30 (trailing h + leading h+1), trailing 8 of row 31.
            for dst, wl in ((kf, kf_writes), (vf, vf_writes)):
                for b in range(B):
                    base = b * DM * H * W2
                    wl.append(nc.sync.dma_start(
                        out=bass.AP(dst.tensor, base,
                                    [[H * W2, DM], [1, FL]]),
                        in_=bass.AP(zpad.tensor, 0, [[16, DM], [1, FL]])).ins)
                    wl.append(nc.sync.dma_start(
                        out=bass.AP(dst.tensor, base + W2 - FL,
                                    [[H * W2, DM], [W2, H - 1], [1, 2 * FL]]),
                        in_=bass.AP(zpad.tensor, 0,
                                    [[512, DM], [16, H - 1],
                                     [1, 2 * FL]])).ins)
                    wl.append(nc.sync.dma_start(
                        out=bass.AP(dst.tensor,
                                    base + (H - 1) * W2 + W2 - FL,
                                    [[H * W2, DM], [1, FL]]),
                        in_=bass.AP(zpad.tensor, 0, [[16, DM], [1, FL]])).ins)

            # conv chunks: ct covers 2 h-rows; flush every FLB chunks
            for b in range(B):
                for hb in range(NCT // FLB):
                    ysb = yst.tile([96, NT * FLB], F32R, tag="ysb",
                                   name="ysb")
                    for ci in range(FLB):
                        ct = hb * FLB + ci
                        h0 = ct * 2
                        yp = cps.tile([96, NT], F32, tag="yp", name="yp")
                        for dy in range(3):
                            off = b * ROWP + (h0 + dy) * WP
                            nc.tensor.matmul(
                                yp[:], wA_sb[dy],
                                _sap(X2, 0, 128, off, [[WP, 2], [1, W]]),
                                start=(dy == 0), stop=False)
                        for dy in range(3):
                            off = b * ROWP + (h0 + dy) * WP + 2
                            nc.tensor.matmul(
                                yp[:], wB_sb[dy],
                                _sap(X2, 0, 64, off, [[WP, 2], [1, W]]),
                                start=False, stop=(dy == 2))
                        # bias + PSUM->SBUF copy, rotating engines
                        dstc = ysb[:, ci * NT:(ci + 1) * NT]
                        if ct % 2 == 0:
                            nc.scalar.activation(dstc, yp[:], AF.Identity,
                                                 bias=bias_sb)
                        else:
                            nc.vector.tensor_scalar_add(dstc, yp[:], bias_sb)
                    # flush q / k / v for this 8-row block
                    hh0 = hb * FLB * 2
                    qf_writes.append(nc.sync.dma_start(
                        out=bass.AP(qfull.tensor, b * PB + hh0 * W,
                                    [[POS, DM], [1, NT * FLB]]),
                        in_=ysb[0:32, :]).ins)
                    for (prow, dst, wl) in ((32, kf, kf_writes),
                                            (64, vf, vf_writes)):
                        wl.append(nc.sync.dma_start(
                            out=bass.AP(dst.tensor,
                                        b * DM * H * W2 + hh0 * W2 + FL,
                                        [[H * W2, DM], [W2, FLB * 2],
                                         [1, W]]),
                            in_=ysb[prow:prow + 32, :]
                            .rearrange("p (h w) -> p h w", h=FLB * 2)).ins)

        # ---------------- attention phase ----------------
        with (
            tc.tile_pool(name="ga", bufs=3) as ga,
            tc.tile_pool(name="ste", bufs=10) as stp,
            tc.tile_pool(name="pst", bufs=2, space="PSUM") as pst,
            tc.tile_pool(name="pvt", bufs=1, space="PSUM") as pvt,
            tc.tile_pool(name="poa", bufs=1, space="PSUM") as poa,
            tc.tile_pool(name="on", bufs=1) as on,
        ):
            o_acc = poa.tile([128, 480], F32, tag="o_acc", name="o_acc")
            # two persistent (v|ones) tiles; ones cols preset once
            vsb = [on.tile([128, 50], F32, tag=f"vsb{i}", name=f"vsb{i}")
                   for i in range(2)]
            for t in vsb:
                nc.vector.memset(_sap(t, 0, 128, 4, [[5, 10], [1, 1]]), 1.0)

            # ste column holding exp(s) for (kt, q): kt even (first of the
            # st pair): q0..768 at half-base 0; kt odd: q0..256 @768,
            # q256..768 @1024 of that st's tile.
            def ste_col(kt, j):
                return (kt % 2) * 768 + j * 128

            def wsum(g, stes, vs):
                # only ONE open psum accumulation region per bank at a
                # time: j outer, kt inner.
                for j in range(6):
                    for kt in range(10):
                        c0 = ste_col(kt, j)
                        nc.tensor.matmul(
                            o_acc[:, g * 30 + j * 5:g * 30 + j * 5 + 5],
                            stes[kt // 2][:, c0:c0 + 128],
                            vs[:, kt * 5:kt * 5 + 5],
                            start=(kt == 0), stop=(kt == 9),
                            skip_group_check=True)

            prev = None  # (g, stes, vs) of previous group
            for n in range(NH):
                for mm in range(2):
                    g = n * 2 + mm
                    gb = cbase + n * S_N + mm * QS
                    qb = qbase + mm * QS

                    k2 = ga.tile([4, HF], F32R, tag="k2", name="k2")
                    i_k = nc.gpsimd.dma_start(
                        out=k2[:],
                        in_=bass.AP(kf.tensor, gb,
                                    [[S_C, 4], [S_H, H], [1, F]]))
                    v2k = ga.tile([4, HF], F32R, tag="v2k", name="v2k")
                    i_v = nc.gpsimd.dma_start(
                        out=v2k[:],
                        in_=bass.AP(vf.tensor, gb,
                                    [[S_C, 4], [S_H, H], [1, F]]))
                    qg = ga.tile([4, HQ], F32R, tag="qg", name="qg")
                    i_q = nc.gpsimd.dma_start(
                        out=qg[:],
                        in_=bass.AP(qfull.tensor, qb + n * CH * POS,
                                    [[POS, 4], [W, H], [1, QS]]))
                    for inst, wl in ((i_k, kf_writes), (i_v, vf_writes),
                                     (i_q, qf_writes)):
                        for wi in wl:
                            add_dep_helper(inst.ins, wi, sync=True,
                                           reason="gather after conv flush")

                    # transpose v2k -> vt [128, 40], then pack into vsb
                    vs = vsb[g % 2]
                    vt = pvt.tile([128, 40], F32R, tag="vt", name="vt")
                    for kt in range(10):
                        nc.tensor.transpose(
                            vt[:, kt * 4:(kt + 1) * 4],
                            v2k[:, kt * 128:(kt + 1) * 128], id_sb)
                    nc.vector.tensor_copy(
                        _sap(vs, 0, 128, 0, [[5, 10], [1, 4]]),
                        _sap(vt, 0, 128, 0, [[4, 10], [1, 4]]))

                    stes = []
                    for st_i in range(5):
                        st = pst.tile([128, 1536], F32, tag="st", name="st")
                        kt0 = st_i * 2
                        kt1 = kt0 + 1
                        nc.tensor.matmul(
                            st[:, 0:512], k2[:, kt0 * 128:(kt0 + 1) * 128],
                            qg[:, 0:512],
                            start=True, stop=True, skip_group_check=True)
                        nc.tensor.matmul(
                            st[:, 512:768], k2[:, kt0 * 128:(kt0 + 1) * 128],
                            qg[:, 512:768],
                            start=True, stop=True, skip_group_check=True)
                        nc.tensor.matmul(
                            st[:, 768:1024], k2[:, kt1 * 128:(kt1 + 1) * 128],
                            qg[:, 0:256],
                            start=True, stop=True, skip_group_check=True)
                        nc.tensor.matmul(
                            st[:, 1024:1536], k2[:, kt1 * 128:(kt1 + 1) * 128],
                            qg[:, 256:768],
                            start=True, stop=True, skip_group_check=True)
                        ste = stp.tile([128, 1536], F32, tag="ste",
                                       name="ste")
                        if st_i == 2:
                            # Schraudolph exp on DVE (int-trick)
                            nc.vector.tensor_scalar(
                                ste[:].bitcast(I32), st[:], SCH_A, SCH_B,
                                ALU.mult, ALU.add)
                        else:
                            nc.scalar.activation(ste[:], st[:], AF.Exp)
                        stes.append(ste)
                        # software pipeline: after this group's first two
                        # score tiles are in flight, run the PREVIOUS
                        # group's weighted sum (its exp is finished by now)
                        if st_i == 1 and prev is not None:
                            wsum(*prev)
                            prev = None
                    prev = (g, stes, vs)
            wsum(*prev)

            o_sb = on.tile([128, 480], F32, tag="o_sb", name="o_sb")
            nc.vector.tensor_copy(o_sb[:], o_acc[:])
            nc.sync.dma_start(out=o_out[:], in_=o_sb[:])

            if DEBUG_DUMPS:
                kdump = nc.dram_tensor("kdump", [KFSZ], F32,
                                       kind="ExternalOutput").ap()
                vdump = nc.dram_tensor("vdump", [KFSZ], F32,
                                       kind="ExternalOutput").ap()
                qdump = nc.dram_tensor("qdump", [DM, POS], F32,
                                       kind="ExternalOutput").ap()
                ik = nc.sync.dma_start(out=kdump[:], in_=kf[:].bitcast(F32))
                iv = nc.sync.dma_start(out=vdump[:], in_=vf[:].bitcast(F32))
                iq = nc.sync.dma_start(out=qdump[:], in_=qfull[:].bitcast(F32))
                for inst, wl in ((ik, kf_writes), (iv, vf_writes),
                                 (iq, qf_writes)):
                    for wi in wl:
                        add_dep_helper(inst.ins, wi, sync=True,
                                       reason="dump after flush")
    nc.finalize()
    return nc


def _build_launch2():
    nc = bacc.Bacc(None, target_bir_lowering=False, debug=False,
                   num_devices=NCORES)
    WH = 2 * QS + 2  # 50 cols with halo
    NPAD2 = (H + 2) * WH  # 1700
    # host prebuilds the 3 dx-shifted copies (96 rows = 3 dx x 32 ch)
    oh3 = nc.dram_tensor("oh3", [96, NPAD2], F32R, kind="ExternalInput").ap()
    w2b = nc.dram_tensor("w2b", [96, 192], F32R, kind="ExternalInput").ap()
    out = nc.dram_tensor("out", [64, H * 2 * QS], F32,
                         kind="ExternalOutput").ap()

    with TileContext(nc) as tc:
        with (
            tc.tile_pool(name="sb", bufs=1) as sb,
            tc.tile_pool(name="ps", bufs=2, space="PSUM") as ps,
        ):
            w2_sb = sb.tile([96, 192], F32R, tag="w2b", name="w2b")
            nc.sync.dma_start(out=w2_sb[:], in_=w2b[:])
            osh = sb.tile([96, NPAD2], F32R, tag="osh", name="osh")
            nc.sync.dma_start(out=osh[:], in_=oh3[:])

            ot = sb.tile([64, H * 2 * QS], F32, tag="ot", name="ot")
            hsz = [10, 10, 10, 2]
            h0 = 0
            for hi, hn in enumerate(hsz):
                nt = hn * WH
                yp = ps.tile([64, 500], F32, tag="yp", name="yp")
                for dy in range(3):
                    off = (h0 + dy) * WH
                    nc.tensor.matmul(
                        yp[:, 0:nt], w2_sb[:, dy * 64:(dy + 1) * 64],
                        _sap(osh, 0, 96, off, [[1, nt]]),
                        start=(dy == 0), stop=(dy == 2))
                nc.vector.tensor_copy(
                    _sap(ot, 0, 64, h0 * 2 * QS, [[2 * QS, hn], [1, 2 * QS]]),
                    _sap(yp, 0, 64, 1, [[WH, hn], [1, 2 * QS]]))
                h0 += hn
            nc.sync.dma_start(out=out[:], in_=ot[:])
    nc.finalize()
    return nc


def _prep_qkv_weights(q_w, q_b, k_w, k_b, v_w, v_b):
    # fold attention scale into q
    sc = CH ** -0.5
    q_w = q_w * sc
    q_b = q_b * sc
    Wc = np.concatenate([q_w, k_w, v_w], axis=0)   # (96, 64, 3, 3)
    bc = np.concatenate([q_b, k_b, v_b], axis=0)   # (96,)
    wA, wB = [], []
    for dy in range(3):
        a = np.zeros((128, 96), np.float32)
        a[0:64, :] = Wc[:, :, dy, 0].T    # dx=-1
        a[64:128, :] = Wc[:, :, dy, 1].T  # dx=0
        wA.append(a)
        wB.append(Wc[:, :, dy, 2].T.copy())  # dx=+1
    return wA, wB, bc


def kernel(x, q_w, q_b, k_w, k_b, v_w, v_b, out_w):
    x = np.asarray(x, np.float32)
    if "l1" not in _CACHE:
        _CACHE["l1"] = _build_launch1()
        _CACHE["l2"] = _build_launch2()
    nc1, nc2 = _CACHE["l1"], _CACHE["l2"]

    wA, wB, cbias = _prep_qkv_weights(
        np.asarray(q_w, np.float32), np.asarray(q_b, np.float32),
        np.asarray(k_w, np.float32), np.asarray(k_b, np.float32),
        np.asarray(v_w, np.float32), np.asarray(v_b, np.float32))
    wblob = np.zeros((128, 640), np.float32)
    for dy in range(3):
        wblob[:, dy * 96:(dy + 1) * 96] = wA[dy]
        wblob[0:64, 288 + dy * 96:288 + (dy + 1) * 96] = wB[dy]
    wblob[0:96, 576] = cbias
    wblob[0:4, 577:581] = np.eye(4, dtype=np.float32)
    wblob[:, 600:610] = 1.0
    zpad = np.zeros((2, 32, 32, 16), np.float32)
    in_maps = []
    for k in range(NCORES):
        b, m0 = k // 4, 2 * (k % 4)
        par = np.array([[b * S_B + m0 * QS, b * PB + m0 * QS, 0, 0]],
                       np.uint32)
        in_maps.append({"xb": x, "zpad": zpad,
                        "par": par, "wblob": wblob})
    res1 = run_bass_kernel_spmd(nc1, in_maps, list(range(NCORES)))

    # assemble o (B, 32, H, W): o_out [128, g*30 + j*5 + c]
    o = np.zeros((B, DM, H, W), np.float32)
    for k in range(NCORES):
        b, m0 = k // 4, 2 * (k % 4)
        oo = res1.results[k]["o_out"].reshape(128, NG, 6, 5)
        num = oo[:, :, :, 0:4]          # (p, g, j, c)
        den = oo[:, :, :, 4]            # (p, g, j)
        on = num / den[:, :, :, None]
        # q = j*128 + p = h*24 + i
        on = on.transpose(1, 3, 2, 0)   # (g, c, j, p)
        on = on.reshape(NG, CH, HQ)     # q = j*128+p
        on = on.reshape(NG, CH, H, QS)  # h = q//24, i = q%24
        for n in range(NH):
            for mm in range(2):
                o[b, n * CH:(n + 1) * CH, :,
                  (m0 + mm) * QS:(m0 + mm + 1) * QS] = on[n * 2 + mm]

    # launch 2: output conv, sharded by (b, column pair)
    ow = np.asarray(out_w, np.float32)
    w2b = np.zeros((96, 192), np.float32)
    for dy in range(3):
        for dx in range(3):
            w2b[dx * 32:(dx + 1) * 32, dy * 64:(dy + 1) * 64] = \
                ow[:, :, dy, dx].T
    WH = 2 * QS + 2
    in_maps2 = []
    for k in range(NCORES):
        b, m0 = k // 4, 2 * (k % 4)
        ohal = np.zeros((DM, H + 2, WH), np.float32)
        c0 = m0 * QS
        lo, hi = max(0, c0 - 1), min(W, c0 + 2 * QS + 1)
        ohal[:, 1:H + 1, (lo - (c0 - 1)):(hi - (c0 - 1))] = o[b, :, :, lo:hi]
        # 3 dx-shifted copies: row block dx needs osh[dx*32+ch, :, w] =
        # ohal[ch, :, w + dx - 1]
        oh3 = np.zeros((96, H + 2, WH), np.float32)
        oh3[32:64] = ohal
        oh3[0:32, :, 1:] = ohal[:, :, :-1]
        oh3[64:96, :, :-1] = ohal[:, :, 1:]
        in_maps2.append({"oh3": oh3.reshape(96, -1), "w2b": w2b})
    res2 = run_bass_kernel_spmd(nc2, in_maps2, list(range(NCORES)))

    out = np.zeros((B, 64, H, W), np.float32)
    for k in range(NCORES):
        b, m0 = k // 4, 2 * (k % 4)
        out[b, :, :, m0 * QS:(m0 + 2) * QS] = \
            res2.results[k]["out"].reshape(64, H, 2 * QS)
    return out


# revision 22
# speedup vs baseline: 1.2618x; 1.0475x over previous
"""Trainium2 Bass kernel for windowed multi-head attention with conv QKV.

Shapes (hardcoded): x (2,64,32,192), D_MODEL=32, N_HEADS=8, c=4, QS=24,
FLANGE=8, F=40, T=192, M=8 blocks. 8 NeuronCores.

Sharding: core k owns batch b=k//4 and block pair m0=2*(k%4); it computes
all 8 heads for its two blocks (16 attention groups) plus its slice of the
final conv (second launch).

Launch 1 structure:
  conv phase: x landed in a zero-padded 194-wide raster (Xp = X2[0:64]);
    X2[64:128] = Xp shifted by +1 col. 6 matmul passes per chunk
    (3 dy taps x {128-part packed dx pair, 64-part dx}), bias added during
    the PSUM->SBUF flush copy, then static DMAs write kf/vf (padded
    208-wide storage) and qfull to DRAM.
  attention phase: per group (head n, block mm): dynamic gpsimd gathers
    k2/v2k/qg straight from kf/vf/qfull (buggy-stride windows, offset
    register cbase/qbase = per-core (b, m0) base). Scores: 20 matmuls
    into PSUM [128,1536] (f32r, keys x queries). exp split: Act engine
    (exact) for 4/5 tiles, DVE Schraudolph int-trick for 1/5. Weighted
    sum reoriented: out[128 queries, 4v+Z] with exp'd scores as the
    stationary and bf16 (v | ones) as the 5-col moving operand,
    accumulated over the 10 key tiles into a persistent PSUM bank
    (one 30-col region per group). Host does the final divide by Z.
"""

import numpy as np
import ml_dtypes
import concourse.bass as bass
import concourse.bacc as bacc
import concourse.mybir as mybir
from concourse.tile import TileContext
from concourse.bass_utils import run_bass_kernel_spmd
from concourse.tile_rust import add_dep_helper

F32 = mybir.dt.float32
F32R = mybir.dt.float32r
I32 = mybir.dt.int32
U32 = mybir.dt.uint32
BF16 = mybir.dt.bfloat16
AF = mybir.ActivationFunctionType
ALU = mybir.AluOpType

NCORES = 8
B, CIN, H, W = 2, 64, 32, 192
DM, NH, CH = 32, 8, 4          # d_model, heads, depth/head
QS, FL, F = 24, 8, 40          # query block, flange, window
M = W // QS                    # 8 blocks
POS = B * H * W                # 12288
PB = H * W                     # 6144 positions per batch
W2 = W + 2 * FL                # 208 storage row
KFSZ = B * DM * H * W2         # 425984 padded k storage (both batches)
# buggy as_strided strides (elements) over the padded storage
S_B, S_N, S_C, S_H = NH * CH * H * W, CH * H * W, H * W, W
HF = H * F                     # 1280 keys per group
HQ = H * QS                    # 768 queries per group
WP = W + 2                     # 194 padded conv raster row
ROWP = (H + 2) * WP            # 6596 raster per batch
NT = 384                       # conv chunk (2 h rows)
NCT = PB // NT                 # 16 chunks per batch
FLB = 4                        # chunks per flush block (8 h rows)
NG = 2 * NH                    # 16 groups per core

# Schraudolph exp constants (round-to-nearest on the f32->i32 convert)
SCH_A = float(2 ** 23 / np.log(2))
SCH_B = float(127 * 2 ** 23 - 366393.0)

_CACHE = {}
DEBUG_DUMPS = False


def _sap(tile, p0, npart, off, dims):
    """Custom strided view of an SBUF/PSUM pool tile."""
    b0 = tile[:]
    ps = int(b0.ap[0][0])
    return bass.AP(b0.tensor, b0.offset + p0 * ps + off, [[ps, npart]] + dims)


def _build_launch1():
    nc = bacc.Bacc(None, target_bir_lowering=False, debug=False,
                   num_devices=NCORES)
    # full x (conv must cover both batches: groups read across the
    # batch boundary via the buggy strides)
    xb = nc.dram_tensor("xb", [B, CIN, H, W], F32R,
                        kind="ExternalInput").ap()
    # blob cols: wA0..2 @ dy*96, wB0..2 @ 288+dy*96 (rows 0:64), bias @576,
    # id4 @ rows 4:8 cols 577:581, ones-bf16 @ col 584 (128 rows)
    wblob = nc.dram_tensor("wblob", [128, 640], F32R,
                           kind="ExternalInput").ap()
    zpad = nc.dram_tensor("zpad", [2, 32, 32, 16], F32R,
                          kind="ExternalInput").ap()
    par = nc.dram_tensor("par", [1, 4], U32, kind="ExternalInput").ap()
    o_out = nc.dram_tensor("o_out", [128, NG * 30], F32,
                           kind="ExternalOutput").ap()

    kf = nc.dram_tensor("kf", [KFSZ], F32R).ap()
    vf = nc.dram_tensor("vf", [KFSZ], F32R).ap()
    qfull = nc.dram_tensor("qfull", [DM, POS], F32R).ap()

    with TileContext(nc) as tc:
      with tc.tile_pool(name="persist", bufs=1) as persist:
        kf_writes, vf_writes, qf_writes = [], [], []
        # ---------------- conv phase ----------------
        with (
            tc.tile_pool(name="xw", bufs=1) as xw,
            tc.tile_pool(name="yst", bufs=3) as yst,
            tc.tile_pool(name="cps", bufs=2, space="PSUM") as cps,
        ):
            par_sb = persist.tile([1, 4], U32, tag="par", name="par")
            nc.sync.dma_start(out=par_sb[:], in_=par[:])
            blob = persist.tile([128, 640], F32R, tag="blob", name="blob")
            nc.sync.dma_start(out=blob[:], in_=wblob[:])
            wA_sb = [blob[:, dy * 96:(dy + 1) * 96] for dy in range(3)]
            wB_sb = [blob[0:64, 288 + dy * 96:288 + (dy + 1) * 96]
                     for dy in range(3)]
            bias_sb = blob[0:96, 576:577].bitcast(F32)
            id_sb = blob[0:4, 577:581]

            # X2 [128, ROWP]: rows 0:64 = padded raster Xp of own batch
            # (row r=h+1 holds [0, x_h, 0]); rows 64:128 = Xp shifted +1.
            X2 = xw.tile([128, 2 * ROWP], F32R, tag="X2", name="X2")
            # per batch raster at b*ROWP: zero pad rows 0 and 33, pad cols
            for b in range(B):
                r0 = b * ROWP
                nc.sync.dma_start(
                    out=_sap(X2, 0, 128, r0, [[1, WP]]),
                    in_=bass.AP(zpad.tensor, 0, [[255, 128], [1, WP]]))
                nc.sync.dma_start(
                    out=_sap(X2, 0, 128, r0 + (H + 1) * WP, [[1, WP]]),
                    in_=bass.AP(zpad.tensor, 0, [[255, 128], [1, WP]]))
                nc.sync.dma_start(
                    out=_sap(X2, 0, 128, r0 + WP, [[WP, H], [1, 1]]),
                    in_=bass.AP(zpad.tensor, 0,
                                [[255, 128], [1, H], [1, 1]]))
                nc.sync.dma_start(
                    out=_sap(X2, 0, 128, r0 + WP + W + 1, [[WP, H], [1, 1]]),
                    in_=bass.AP(zpad.tensor, 0,
                                [[255, 128], [1, H], [1, 1]]))
                for hh in range(2):
                    xs = xb[b]
                    nc.sync.dma_start(
                        out=_sap(X2, 0, 64, r0 + (hh * 16 + 1) * WP + 1,
                                 [[WP, 16], [1, W]]),
                        in_=bass.AP(xs.tensor, xs.offset + hh * 16 * W,
                                    [[H * W, 64], [W, 16], [1, W]]))
                    # shifted copy of this half (plus row 0 / row 33 pads)
                    c0 = r0 + hh * 16 * WP + (0 if hh == 0 else WP)
                    cn = 17 * WP if hh == 0 else (17 * WP - (1 if b == 1 else 0))
                    nc.vector.tensor_copy(
                        _sap(X2, 64, 64, c0, [[1, cn]]),
                        _sap(X2, 0, 64, c0 + 1, [[1, cn]]))

            # registers for dynamic gathers (gpsimd)
            r0 = nc.gpsimd.alloc_register("cb")
            nc.gpsimd.reg_load(r0, par_sb[0:1, 0:1])
            cbase = nc.snap(r0, min_val=0, max_val=300000)
            r1 = nc.gpsimd.alloc_register("qb")
            nc.gpsimd.reg_load(r1, par_sb[0:1, 1:2])
            qbase = nc.snap(r1, min_val=0, max_val=300000)

            # flange zeros in kf/vf, per batch & tensor:
            #   lead 8 of row 0 (per ch), 16-wide run at col 200 of rows
            #   0..30 (trailing h + leading h+1), trailing 8 of row 31.
            for dst, wl in ((kf, kf_writes), (vf, vf_writes)):
                for b in range(B):
                    base = b * DM * H * W2
                    wl.append(nc.sync.dma_start(
                        out=bass.AP(dst.tensor, base,
                                    [[H * W2, DM], [1, FL]]),
                        in_=bass.AP(zpad.tensor, 0, [[16, DM], [1, FL]])).ins)
                    wl.append(nc.sync.dma_start(
                        out=bass.AP(dst.tensor, base + W2 - FL,
                                    [[H * W2, DM], [W2, H - 1], [1, 2 * FL]]),
                        in_=bass.AP(zpad.tensor, 0,
                                    [[512, DM], [16, H - 1],
                                     [1, 2 * FL]])).ins)
                    wl.append(nc.sync.dma_start(
                        out=bass.AP(dst.tensor,
                                    base + (H - 1) * W2 + W2 - FL,
                                    [[H * W2, DM], [1, FL]]),
                        in_=bass.AP(zpad.tensor, 0, [[16, DM], [1, FL]])).ins)

            # conv chunks: ct covers 2 h-rows; flush every FLB chunks
            for b in range(B):
                for hb in range(NCT // FLB):
                    ysb = yst.tile([96, NT * FLB], F32R, tag="ysb",
                                   name="ysb")
                    for ci in range(FLB):
                        ct = hb * FLB + ci
                        h0 = ct * 2
                        yp = cps.tile([96, NT], F32, tag="yp", name="yp")
                        for dy in range(3):
                            off = b * ROWP + (h0 + dy) * WP
                            nc.tensor.matmul(
                                yp[:], wA_sb[dy],
                                _sap(X2, 0, 128, off, [[WP, 2], [1, W]]),
                                start=(dy == 0), stop=False)
                        for dy in range(3):
                            off = b * ROWP + (h0 + dy) * WP + 2
                            nc.tensor.matmul(
                                yp[:], wB_sb[dy],
                                _sap(X2, 0, 64, off, [[WP, 2], [1, W]]),
                                start=False, stop=(dy == 2))
                        # bias + PSUM->SBUF copy, rotating engines
                        dstc = ysb[:, ci * NT:(ci + 1) * NT]
                        if ct % 2 == 0:
                            nc.scalar.activation(dstc, yp[:], AF.Identity,
                                                 bias=bias_sb)
                        else:
                            nc.vector.tensor_scalar_add(dstc, yp[:], bias_sb)
                    # flush q / k / v for this 8-row block
                    hh0 = hb * FLB * 2
                    qf_writes.append(nc.sync.dma_start(
                        out=bass.AP(qfull.tensor, b * PB + hh0 * W,
                                    [[POS, DM], [1, NT * FLB]]),
                        in_=ysb[0:32, :]).ins)
                    for (prow, dst, wl) in ((32, kf, kf_writes),
                                            (64, vf, vf_writes)):
                        wl.append(nc.sync.dma_start(
                            out=bass.AP(dst.tensor,
                                        b * DM * H * W2 + hh0 * W2 + FL,
                                        [[H * W2, DM], [W2, FLB * 2],
                                         [1, W]]),
                            in_=ysb[prow:prow + 32, :]
                            .rearrange("p (h w) -> p h w", h=FLB * 2)).ins)

        # ---------------- attention phase ----------------
        with (
            tc.tile_pool(name="ga", bufs=3) as ga,
            tc.tile_pool(name="ste", bufs=10) as stp,
            tc.tile_pool(name="pst", bufs=2, space="PSUM") as pst,
            tc.tile_pool(name="pvt", bufs=1, space="PSUM") as pvt,
            tc.tile_pool(name="poa", bufs=1, space="PSUM") as poa,
            tc.tile_pool(name="on", bufs=1) as on,
        ):
            o_acc = poa.tile([128, 480], F32, tag="o_acc", name="o_acc")
            # two persistent (v|ones) tiles; ones cols preset once
            vsb = [on.tile([128, 50], F32, tag=f"vsb{i}", name=f"vsb{i}")
                   for i in range(2)]
            for t in vsb:
                nc.vector.memset(_sap(t, 0, 128, 4, [[5, 10], [1, 1]]), 1.0)

            # ste column holding exp(s) for (kt, q): kt even (first of the
            # st pair): q0..768 at half-base 0; kt odd: q0..256 @768,
            # q256..768 @1024 of that st's tile.
            def ste_col(kt, j):
                return (kt % 2) * 768 + j * 128

            def wsum(g, stes, vs):
                # only ONE open psum accumulation region per bank at a
                # time: j outer, kt inner.
                for j in range(6):
                    for kt in range(10):
                        c0 = ste_col(kt, j)
                        nc.tensor.matmul(
                            o_acc[:, g * 30 + j * 5:g * 30 + j * 5 + 5],
                            stes[kt // 2][:, c0:c0 + 128],
                            vs[:, kt * 5:kt * 5 + 5],
                            start=(kt == 0), stop=(kt == 9),
                            skip_group_check=True)

            prev = None  # (g, stes, vs) of previous group
            for n in range(NH):
                for mm in range(2):
                    g = n * 2 + mm
                    gb = cbase + n * S_N + mm * QS
                    qb = qbase + mm * QS

                    k2 = ga.tile([4, HF], F32R, tag="k2", name="k2")
                    i_k = nc.gpsimd.dma_start(
                        out=k2[:],
                        in_=bass.AP(kf.tensor, gb,
                                    [[S_C, 4], [S_H, H], [1, F]]))
                    v2k = ga.tile([4, HF], F32R, tag="v2k", name="v2k")
                    i_v = nc.gpsimd.dma_start(
                        out=v2k[:],
                        in_=bass.AP(vf.tensor, gb,
                                    [[S_C, 4], [S_H, H], [1, F]]))
                    qg = ga.tile([4, HQ], F32R, tag="qg", name="qg")
                    i_q = nc.gpsimd.dma_start(
                        out=qg[:],
                        in_=bass.AP(qfull.tensor, qb + n * CH * POS,
                                    [[POS, 4], [W, H], [1, QS]]))
                    for inst, wl in ((i_k, kf_writes), (i_v, vf_writes),
                                     (i_q, qf_writes)):
                        for wi in wl:
                            add_dep_helper(inst.ins, wi, sync=True,
                                           reason="gather after conv flush")

                    # transpose v2k -> vt [128, 40], then pack into vsb
                    vs = vsb[g % 2]
                    vt = pvt.tile([128, 40], F32R, tag="vt", name="vt")
                    for kt in range(10):
                        nc.tensor.transpose(
                            vt[:, kt * 4:(kt + 1) * 4],
                            v2k[:, kt * 128:(kt + 1) * 128], id_sb)
                    nc.vector.tensor_copy(
                        _sap(vs, 0, 128, 0, [[5, 10], [1, 4]]),
                        _sap(vt, 0, 128, 0, [[4, 10], [1, 4]]))

                    stes = []
                    for st_i in range(5):
                        st = pst.tile([128, 1536], F32, tag="st", name="st")
                        kt0 = st_i * 2
                        kt1 = kt0 + 1
                        nc.tensor.matmul(
                            st[:, 0:512], k2[:, kt0 * 128:(kt0 + 1) * 128],
                            qg[:, 0:512],
                            start=True, stop=True, skip_group_check=True)
                        nc.tensor.matmul(
                            st[:, 512:768], k2[:, kt0 * 128:(kt0 + 1) * 128],
                            qg[:, 512:768],
                            start=True, stop=True, skip_group_check=True)
                        nc.tensor.matmul(
                            st[:, 768:1024], k2[:, kt1 * 128:(kt1 + 1) * 128],
                            qg[:, 0:256],
                            start=True, stop=True, skip_group_check=True)
                        nc.tensor.matmul(
                            st[:, 1024:1536], k2[:, kt1 * 128:(kt1 + 1) * 128],
                            qg[:, 256:768],
                            start=True, stop=True, skip_group_check=True)
                        ste = stp.tile([128, 1536], F32, tag="ste",
                                       name="ste")
                        if st_i in (1, 3):
                            # Schraudolph exp on DVE (int-trick)
                            nc.vector.tensor_scalar(
                                ste[:].bitcast(I32), st[:], SCH_A, SCH_B,
                                ALU.mult, ALU.add)
                        else:
                            nc.scalar.activation(ste[:], st[:], AF.Exp)
                        stes.append(ste)
                        # software pipeline: after this group's first two
                        # score tiles are in flight, run the PREVIOUS
                        # group's weighted sum (its exp is finished by now)
                        if st_i == 1 and prev is not None:
                            wsum(*prev)
                            prev = None
                    prev = (g, stes, vs)
            wsum(*prev)

            o_sb = on.tile([128, 480], F32, tag="o_sb", name="o_sb")
            nc.vector.tensor_copy(o_sb[:], o_acc[:])
            nc.sync.dma_start(out=o_out[:], in_=o_sb[:])

            if DEBUG_DUMPS:
                kdump = nc.dram_tensor("kdump", [KFSZ], F32,
                                       kind="ExternalOutput").ap()
                vdump = nc.dram_tensor("vdump", [KFSZ], F32,
                                       kind="ExternalOutput").ap()
                qdump = nc.dram_tensor("qdump", [DM, POS], F32,
                                       kind="ExternalOutput").ap()
                ik = nc.sync.dma_start(out=kdump[:], in_=kf[:].bitcast(F32))
                iv = nc.sync.dma_start(out=vdump[:], in_=vf[:].bitcast(F32))
                iq = nc.sync.dma_start(out=qdump[:], in_=qfull[:].bitcast(F32))
                for inst, wl in ((ik, kf_writes), (iv, vf_writes),
                                 (iq, qf_writes)):
                    for wi in wl:
                        add_dep_helper(inst.ins, wi, sync=True,
                                       reason="dump after flush")
    nc.finalize()
    return nc


def _build_launch2():
    nc = bacc.Bacc(None, target_bir_lowering=False, debug=False,
                   num_devices=NCORES)
    WH = 2 * QS + 2  # 50 cols with halo
    NPAD2 = (H + 2) * WH  # 1700
    # host prebuilds the 3 dx-shifted copies (96 rows = 3 dx x 32 ch)
    oh3 = nc.dram_tensor("oh3", [96, NPAD2], F32R, kind="ExternalInput").ap()
    w2b = nc.dram_tensor("w2b", [96, 192], F32R, kind="ExternalInput").ap()
    out = nc.dram_tensor("out", [64, H * 2 * QS], F32,
                         kind="ExternalOutput").ap()

    with TileContext(nc) as tc:
        with (
            tc.tile_pool(name="sb", bufs=1) as sb,
            tc.tile_pool(name="ps", bufs=2, space="PSUM") as ps,
        ):
            w2_sb = sb.tile([96, 192], F32R, tag="w2b", name="w2b")
            nc.sync.dma_start(out=w2_sb[:], in_=w2b[:])
            osh = sb.tile([96, NPAD2], F32R, tag="osh", name="osh")
            nc.sync.dma_start(out=osh[:], in_=oh3[:])

            ot = sb.tile([64, H * 2 * QS], F32, tag="ot", name="ot")
            hsz = [10, 10, 10, 2]
            h0 = 0
            for hi, hn in enumerate(hsz):
                nt = hn * WH
                yp = ps.tile([64, 500], F32, tag="yp", name="yp")
                for dy in range(3):
                    off = (h0 + dy) * WH
                    nc.tensor.matmul(
                        yp[:, 0:nt], w2_sb[:, dy * 64:(dy + 1) * 64],
                        _sap(osh, 0, 96, off, [[1, nt]]),
                        start=(dy == 0), stop=(dy == 2))
                nc.vector.tensor_copy(
                    _sap(ot, 0, 64, h0 * 2 * QS, [[2 * QS, hn], [1, 2 * QS]]),
                    _sap(yp, 0, 64, 1, [[WH, hn], [1, 2 * QS]]))
                h0 += hn
            nc.sync.dma_start(out=out[:], in_=ot[:])
    nc.finalize()
    return nc


def _prep_qkv_weights(q_w, q_b, k_w, k_b, v_w, v_b):
    # fold attention scale into q
    sc = CH ** -0.5
    q_w = q_w * sc
    q_b = q_b * sc
    Wc = np.concatenate([q_w, k_w, v_w], axis=0)   # (96, 64, 3, 3)
    bc = np.concatenate([q_b, k_b, v_b], axis=0)   # (96,)
    wA, wB = [], []
    for dy in range(3):
        a = np.zeros((128, 96), np.float32)
        a[0:64, :] = Wc[:, :, dy, 0].T    # dx=-1
        a[64:128, :] = Wc[:, :, dy, 1].T  # dx=0
        wA.append(a)
        wB.append(Wc[:, :, dy, 2].T.copy())  # dx=+1
    return wA, wB, bc


def kernel(x, q_w, q_b, k_w, k_b, v_w, v_b, out_w):
    x = np.asarray(x, np.float32)
    if "l1" not in _CACHE:
        _CACHE["l1"] = _build_launch1()
        _CACHE["l2"] = _build_launch2()
    nc1, nc2 = _CACHE["l1"], _CACHE["l2"]

    wA, wB, cbias = _prep_qkv_weights(
        np.asarray(q_w, np.float32), np.asarray(q_b, np.float32),
        np.asarray(k_w, np.float32), np.asarray(k_b, np.float32),
        np.asarray(v_w, np.float32), np.asarray(v_b, np.float32))
    wblob = np.zeros((128, 640), np.float32)
    for dy in range(3):
        wblob[:, dy * 96:(dy + 1) * 96] = wA[dy]
        wblob[0:64, 288 + dy * 96:288 + (dy + 1) * 96] = wB[dy]
    wblob[0:96, 576] = cbias
    wblob[0:4, 577:581] = np.eye(4, dtype=np.float32)
    wblob[:, 600:610] = 1.0
    zpad = np.zeros((2, 32, 32, 16), np.float32)
    in_maps = []
    for k in range(NCORES):
        b, m0 = k // 4, 2 * (k % 4)
        par = np.array([[b * S_B + m0 * QS, b * PB + m0 * QS, 0, 0]],
                       np.uint32)
        in_maps.append({"xb": x, "zpad": zpad,
                        "par": par, "wblob": wblob})
    res1 = run_bass_kernel_spmd(nc1, in_maps, list(range(NCORES)))

    # assemble o (B, 32, H, W): o_out [128, g*30 + j*5 + c]
    o = np.zeros((B, DM, H, W), np.float32)
    for k in range(NCORES):
        b, m0 = k // 4, 2 * (k % 4)
        oo = res1.results[k]["o_out"].reshape(128, NG, 6, 5)
        num = oo[:, :, :, 0:4]          # (p, g, j, c)
        den = oo[:, :, :, 4]            # (p, g, j)
        on = num / den[:, :, :, None]
        # q = j*128 + p = h*24 + i
        on = on.transpose(1, 3, 2, 0)   # (g, c, j, p)
        on = on.reshape(NG, CH, HQ)     # q = j*128+p
        on = on.reshape(NG, CH, H, QS)  # h = q//24, i = q%24
        for n in range(NH):
            for mm in range(2):
                o[b, n * CH:(n + 1) * CH, :,
                  (m0 + mm) * QS:(m0 + mm + 1) * QS] = on[n * 2 + mm]

    # launch 2: output conv, sharded by (b, column pair)
    ow = np.asarray(out_w, np.float32)
    w2b = np.zeros((96, 192), np.float32)
    for dy in range(3):
        for dx in range(3):
            w2b[dx * 32:(dx + 1) * 32, dy * 64:(dy + 1) * 64] = \
                ow[:, :, dy, dx].T
    WH = 2 * QS + 2
    in_maps2 = []
    for k in range(NCORES):
        b, m0 = k // 4, 2 * (k % 4)
        ohal = np.zeros((DM, H + 2, WH), np.float32)
        c0 = m0 * QS
        lo, hi = max(0, c0 - 1), min(W, c0 + 2 * QS + 1)
        ohal[:, 1:H + 1, (lo - (c0 - 1)):(hi - (c0 - 1))] = o[b, :, :, lo:hi]
        # 3 dx-shifted copies: row block dx needs osh[dx*32+ch, :, w] =
        # ohal[ch, :, w + dx - 1]
        oh3 = np.zeros((96, H + 2, WH), np.float32)
        oh3[32:64] = ohal
        oh3[0:32, :, 1:] = ohal[:, :, :-1]
        oh3[64:96, :, :-1] = ohal[:, :, 1:]
        in_maps2.append({"oh3": oh3.reshape(96, -1), "w2b": w2b})
    res2 = run_bass_kernel_spmd(nc2, in_maps2, list(range(NCORES)))

    out = np.zeros((B, 64, H, W), np.float32)
    for k in range(NCORES):
        b, m0 = k // 4, 2 * (k % 4)
        out[b, :, :, m0 * QS:(m0 + 2) * QS] = \
            res2.results[k]["out"].reshape(64, H, 2 * QS)
    return out
